# revision 11
# baseline (speedup 1.0000x reference)
"""Trainium2 Bass kernel for nn_DerivNet2D_v2 (quadratic-feature MLP fwd + 2
directional derivatives).

Math (feature-major, per sample n):
  h1 = W5 @ [x0^2; x1^2; x0; x1; 1]             (1024, nx)   b1 folded in
  z1 = tanh(h1);  e1 = z1^2 - 1                 (= -sech^2(h1))
  h2 = w2 @ e1 + (b2 + rowsum(w2));  z2 = tanh(h2)
  e2 = z2^2 - 1;  y = w3 @ e2 + (b3 + sum(w3))  (bias-fold of the +1)
  gt = e2 * (z2 * w3s),  w3s = -4*w3            (= G = 4 w3 z2 (1-z2^2))
  v  = w2^T-contraction: v[i,n] = sum_j w2[j,i] gt[j,n]
  qt = e1 * z1  (= -z1(1-z1^2));  qv = qt * v
  d-reduce: psd[0:4] = sum_{it<4} wd_it^T qv_it; psd[32:36] = sum_{it>=4}
  y-reduce: psd[64] = sum_{jt<4} w3_jt^T e2_jt; psd[96] = rest
  tt = psd * xx  (xx rows: {0-3,32-35}=[x0,1,x1,1], {64,96}=1, else 0)
  f  = wf^T @ tt -> [ -dydx1, dydx2, y-b3' ];  fs = f + [0,0,b3'] -> outputs

Engine placement: big matmuls (h2, v) + all reductions on PE; tanh/
final-copy on ACT; sq/e1/e2/zw/qv/tt on DVE; qt/gt on GPSIMD; SWDGE out.
Small matmuls packed with tile_position: h1 row-tiled 2x (K=5), d/y reduces
col-tiled 4x (M<=4).  Software pipeline: front_a (h1 stage) runs two chunks
ahead so the PE never waits on the ACT tanh queue at chunk boundaries.

Sharding: pure data-parallel over 8 cores along batch; weights replicated.
"""

import numpy as np
from contextlib import ExitStack

import concourse.bass as bass
import concourse.tile as tile
from concourse import bacc, mybir
from concourse.bass_utils import run_bass_kernel_spmd

F32 = mybir.dt.float32
F16 = mybir.dt.float16
AF = mybir.ActivationFunctionType
ALU = mybir.AluOpType

NX = 32768
H = 1024
N_CORES = 8
NXL = NX // N_CORES  # 4096 per core
JT = H // 128        # 8 feature tiles of 128

# Fallback flags (set True only if HW misbehaves)
NO_ROW_TILE_H1 = False   # serialize h1 matmuls in 128x128 mode
NO_COL_TILE_DY = False   # serialize d/y reduce matmuls
QT_ON_DVE = False        # compute qt on DVE instead of GPSIMD
GT_ON_DVE = False        # compute gt on DVE instead of GPSIMD


def build_program(nxl: int, C: int):
    nch = nxl // C
    nc = bacc.Bacc("TRN2", target_bir_lowering=False, debug=False,
                   enable_asserts=False)

    # ---- DRAM I/O ----
    xr = nc.dram_tensor("xr", (5, nxl), F16, kind="ExternalInput").ap()
    xq = nc.dram_tensor("xq", (4, nxl), F16, kind="ExternalInput").ap()
    wh1 = nc.dram_tensor("wh1", (5, JT * 128), F16, kind="ExternalInput").ap()
    wh2 = nc.dram_tensor("wh2", (128, JT * H), F16, kind="ExternalInput").ap()
    wv = nc.dram_tensor("wv", (128, JT * H), F16, kind="ExternalInput").ap()
    wy = nc.dram_tensor("wy", (128, JT), F16, kind="ExternalInput").ap()
    wd = nc.dram_tensor("wd", (128, 4 * JT), F16, kind="ExternalInput").ap()
    wf = nc.dram_tensor("wf", (128, 3), F16, kind="ExternalInput").ap()
    b2t = nc.dram_tensor("b2t", (128, JT), F32, kind="ExternalInput").ap()
    b3v = nc.dram_tensor("b3v", (4, 1), F32, kind="ExternalInput").ap()
    w3s = nc.dram_tensor("w3s", (128, JT), F32, kind="ExternalInput").ap()

    outy = nc.dram_tensor("outy", (1, nxl), F32, kind="ExternalOutput").ap()
    outd2 = nc.dram_tensor("outd2", (1, nxl), F32, kind="ExternalOutput").ap()
    outm1 = nc.dram_tensor("outm1", (1, nxl), F32, kind="ExternalOutput").ap()

    with tile.TileContext(nc) as tc, ExitStack() as ctx:
        # ---- persistent tiles ----
        wpool = ctx.enter_context(tc.tile_pool(name="weights", bufs=1))
        s_wh2 = wpool.tile([128, JT * H], F16, tag="wh2")
        s_wv = wpool.tile([128, JT * H], F16, tag="wv")
        s_wh1 = wpool.tile([128, JT * 128], F16, tag="wh1")
        s_wy = wpool.tile([128, JT], F16, tag="wy")
        s_wd = wpool.tile([128, 4 * JT], F16, tag="wd")
        s_wf = wpool.tile([128, 3], F16, tag="wf")
        s_b2 = wpool.tile([128, JT], F32, tag="b2")
        s_b3 = wpool.tile([4, 1], F32, tag="b3")
        s_w3s = wpool.tile([128, JT], F32, tag="w3s")
        r4 = wpool.tile([128, nxl], F16, tag="r4")
        xx = wpool.tile([128, nxl], F16, tag="xx")

        # prewarm the ACT table (tanh) off the critical path
        warm = wpool.tile([128, 16], F32, tag="warm")
        nc.gpsimd.memset(warm[:], 0.0)
        nc.scalar.activation(warm[:], warm[:], AF.Tanh)

        # ---- input DMAs (emission order approximates priority) ----
        nc.sync.dma_start(r4[0:5, :], xr[:])
        nc.sync.dma_start(s_wh1[0:5, :], wh1[:])
        nc.sync.dma_start(s_b2[:], b2t[:])
        nc.sync.dma_start(s_w3s[:], w3s[:])
        nc.sync.dma_start(s_wy[:], wy[:])
        nc.sync.dma_start(s_wd[:], wd[:])
        nc.sync.dma_start(s_wf[:], wf[:])
        nc.sync.dma_start(s_b3[:], b3v[:])
        # replicate x rows for the second h1 row-tile position
        nc.sync.dma_start(r4[64:69, :], r4[0:5, :])
        nc.sync.dma_start(s_wh1[64:69, :], s_wh1[0:5, :])
        for jt in range(JT):
            nc.sync.dma_start(s_wh2[:, jt * H:(jt + 1) * H],
                              wh2[:, jt * H:(jt + 1) * H])
        # xx: zero, then fill rows {0-3, 32-35} with [x0,1,x1,1], {64,96}=1
        nc.gpsimd.memset(xx[:], 0.0)
        nc.sync.dma_start(xx[0:4, :], xq[:])
        nc.sync.dma_start(xx[32:36, :], xx[0:4, :])
        nc.gpsimd.memset(xx[64:65, :], 1.0)
        nc.gpsimd.memset(xx[96:97, :], 1.0)
        for it in range(JT):
            nc.sync.dma_start(s_wv[:, it * H:(it + 1) * H],
                              wv[:, it * H:(it + 1) * H])

        # ---- pools ----
        p_z1sq = ctx.enter_context(tc.tile_pool(name="z1sq", bufs=3))
        p_qt = ctx.enter_context(tc.tile_pool(name="qt", bufs=3))
        p_z2sq = ctx.enter_context(tc.tile_pool(name="z2sq", bufs=2))
        p_gt = ctx.enter_context(tc.tile_pool(name="gt", bufs=2))
        p_qv = ctx.enter_context(tc.tile_pool(name="qv", bufs=2))
        p_z1 = ctx.enter_context(tc.tile_pool(name="z1", bufs=10))
        p_z2 = ctx.enter_context(tc.tile_pool(name="z2", bufs=6))
        p_zw = ctx.enter_context(tc.tile_pool(name="zw", bufs=4))
        p_tt = ctx.enter_context(tc.tile_pool(name="tt", bufs=2))
        p_fs = ctx.enter_context(tc.tile_pool(name="fs", bufs=2))
        p_big = ctx.enter_context(tc.tile_pool(name="bigps", bufs=4, space="PSUM"))
        p_h1 = ctx.enter_context(tc.tile_pool(name="h1ps", bufs=2, space="PSUM"))
        p_dy = ctx.enter_context(tc.tile_pool(name="dyps", bufs=1, space="PSUM"))
        p_f = ctx.enter_context(tc.tile_pool(name="fps", bufs=1, space="PSUM"))

        # d/y reduction accumulator bank: col-tiled matmuls write partition
        # groups {0-3, 32-35, 64, 96}; everything else must stay 0 (the f
        # matmul reads all 128 partitions).
        psdy = p_dy.tile([128, C], F32, tag="psdy")
        nc.vector.memset(psdy[:], 0.0)

        # PE clock prewarm (HAM gate holds PE at 1.2 GHz for ~3.4us)
        wtile = wpool.tile([128, C], F16, tag="warmw")
        nc.gpsimd.memset(wtile[:], 0.0)
        def warmup(n):
            psw = p_big.tile([128, C], F32, tag="big")
            for _ in range(n):
                nc.tensor.matmul(psw[:], wtile[:, 0:128], wtile[:],
                                 start=True, stop=True)

        def front_a(ch):
            """h1 matmuls (row-tiled 2x) -> tanh -> e1 (DVE), qt (GPSIMD)."""
            cs = slice(ch * C, (ch + 1) * C)
            z1s = []
            if NO_ROW_TILE_H1:
                for kt in range(JT):
                    ps = p_h1.tile([128, C], F32, tag="h1")
                    nc.tensor.matmul(ps[:], s_wh1[0:5, kt * 128:(kt + 1) * 128],
                                     r4[0:5, cs], start=True, stop=True)
                    z1 = p_z1.tile([128, C], F16, tag="z1")
                    nc.scalar.activation(z1[:], ps[:], AF.Tanh)
                    z1s.append(z1)
            else:
                for w in range(4):
                    kt0, kt1 = 2 * w, 2 * w + 1
                    psA = p_h1.tile([128, C], F32, tag="h1")
                    psB = p_h1.tile([128, C], F32, tag="h1")
                    nc.tensor.matmul(psA[:], s_wh1[0:5, kt0 * 128:(kt0 + 1) * 128],
                                     r4[0:5, cs], start=True, stop=True,
                                     tile_position=(0, 0))
                    nc.tensor.matmul(psB[:], s_wh1[64:69, kt1 * 128:(kt1 + 1) * 128],
                                     r4[64:69, cs], start=True, stop=True,
                                     tile_position=(64, 0))
                    zA = p_z1.tile([128, C], F16, tag="z1")
                    zB = p_z1.tile([128, C], F16, tag="z1")
                    nc.scalar.activation(zA[:], psA[:], AF.Tanh)
                    nc.scalar.activation(zB[:], psB[:], AF.Tanh)
                    z1s += [zA, zB]
            e1 = p_z1sq.tile([128, JT * C], F16, tag="e1")
            qt = p_qt.tile([128, JT * C], F16, tag="qt")
            qt_eng = nc.vector if QT_ON_DVE else nc.gpsimd
            for kt in range(JT):
                ks = slice(kt * C, (kt + 1) * C)
                sq = p_zw.tile([128, C], F16, tag="zw")
                nc.vector.tensor_mul(sq[:], z1s[kt][:], z1s[kt][:])
                nc.vector.tensor_scalar_sub(e1[:, ks], sq[:], 1.0)
                qt_eng.tensor_mul(qt[:, ks], e1[:, ks], z1s[kt][:])
            return cs, e1, qt

        def front_b(ch, st_a):
            """h2 -> z2 -> e2, gt."""
            cs, e1, qt = st_a
            e2 = p_z2sq.tile([128, JT * C], F16, tag="e2")
            gt = p_gt.tile([128, JT * C], F16, tag="gt")
            gt_eng = nc.vector if GT_ON_DVE else nc.gpsimd
            for jt in range(JT):
                js = slice(jt * C, (jt + 1) * C)
                ps = p_big.tile([128, C], F32, tag="big")
                for kt in range(JT):
                    nc.tensor.matmul(
                        ps[:],
                        s_wh2[:, jt * H + kt * 128:jt * H + (kt + 1) * 128],
                        e1[:, kt * C:(kt + 1) * C],
                        start=(kt == 0), stop=(kt == JT - 1))
                z2 = p_z2.tile([128, C], F16, tag="z2")
                nc.scalar.activation(z2[:], ps[:], AF.Tanh,
                                     bias=s_b2[:, jt:jt + 1])
                sq = p_zw.tile([128, C], F16, tag="zw")
                nc.vector.tensor_mul(sq[:], z2[:], z2[:])
                nc.vector.tensor_scalar_sub(e2[:, js], sq[:], 1.0)
                zw = p_zw.tile([128, C], F16, tag="zw")
                nc.vector.tensor_scalar_mul(zw[:], z2[:], s_w3s[:, jt:jt + 1])
                gt_eng.tensor_mul(gt[:, js], e2[:, js], zw[:])
            return cs, qt, gt, e2

        def v_part(st_b):
            """v matmuls (pair-interleaved) and qv."""
            cs, qt, gt, e2 = st_b
            qv = p_qv.tile([128, JT * C], F16, tag="qv")
            for p in range(4):
                it0, it1 = 2 * p, 2 * p + 1
                ps0 = p_big.tile([128, C], F32, tag="big")
                ps1 = p_big.tile([128, C], F32, tag="big")
                for jt in range(JT):
                    nc.tensor.matmul(
                        ps0[:],
                        s_wv[:, it0 * H + jt * 128:it0 * H + (jt + 1) * 128],
                        gt[:, jt * C:(jt + 1) * C],
                        start=(jt == 0), stop=(jt == JT - 1))
                    nc.tensor.matmul(
                        ps1[:],
                        s_wv[:, it1 * H + jt * 128:it1 * H + (jt + 1) * 128],
                        gt[:, jt * C:(jt + 1) * C],
                        start=(jt == 0), stop=(jt == JT - 1))
                nc.vector.tensor_mul(qv[:, it0 * C:(it0 + 1) * C],
                                     qt[:, it0 * C:(it0 + 1) * C], ps0[:])
                nc.vector.tensor_mul(qv[:, it1 * C:(it1 + 1) * C],
                                     qt[:, it1 * C:(it1 + 1) * C], ps1[:])
            return cs, qv, e2

        def dy_part(st_v):
            """col-tiled d/y reduces -> tt."""
            cs, qv, e2 = st_v
            if NO_COL_TILE_DY:
                for w in range(JT):
                    nc.tensor.matmul(psdy[0:4, :], s_wd[:, 4 * w:4 * w + 4],
                                     qv[:, w * C:(w + 1) * C],
                                     start=(w == 0), stop=(w == JT - 1),
                                     skip_group_check=True)
                for w in range(JT):
                    nc.tensor.matmul(psdy[64:65, :], s_wy[:, w:w + 1],
                                     e2[:, w * C:(w + 1) * C],
                                     start=(w == 0), stop=(w == JT - 1),
                                     skip_group_check=True)
            else:
                for w in range(4):
                    nc.tensor.matmul(psdy[0:4, :], s_wd[:, 4 * w:4 * w + 4],
                                     qv[:, w * C:(w + 1) * C],
                                     start=(w == 0), stop=(w == 3),
                                     tile_position=(0, 0),
                                     skip_group_check=True)
                    nc.tensor.matmul(psdy[32:36, :],
                                     s_wd[:, 4 * (4 + w):4 * (4 + w) + 4],
                                     qv[:, (4 + w) * C:(5 + w) * C],
                                     start=(w == 0), stop=(w == 3),
                                     tile_position=(0, 32),
                                     skip_group_check=True)
                    nc.tensor.matmul(psdy[64:65, :], s_wy[:, w:w + 1],
                                     e2[:, w * C:(w + 1) * C],
                                     start=(w == 0), stop=(w == 3),
                                     tile_position=(0, 64),
                                     skip_group_check=True)
                    nc.tensor.matmul(psdy[96:97, :], s_wy[:, 4 + w:5 + w],
                                     e2[:, (4 + w) * C:(5 + w) * C],
                                     start=(w == 0), stop=(w == 3),
                                     tile_position=(0, 96),
                                     skip_group_check=True)
            tt = p_tt.tile([128, C], F16, tag="tt")
            nc.vector.tensor_mul(tt[:], psdy[:], xx[:, cs])
            return cs, tt

        def f_part(st_d):
            """final combine matmul -> fs -> output DMAs."""
            cs, tt = st_d
            psf = p_f.tile([4, C], F32, tag="f")
            nc.tensor.matmul(psf[0:3, :], s_wf[:], tt[:],
                             start=True, stop=True)
            fs = p_fs.tile([3, C], F32, tag="fs")
            nc.scalar.activation(fs[:], psf[0:3, :], AF.Identity,
                                 bias=s_b3[0:3, 0:1])
            nc.gpsimd.dma_start(outm1[0:1, cs], fs[0:1, :])
            nc.gpsimd.dma_start(outd2[0:1, cs], fs[1:2, :])
            nc.gpsimd.dma_start(outy[0:1, cs], fs[2:3, :])

        # ---- prologue: warmup + h1 for chunks 0,1 ----
        warmup(4)
        fa = {}
        fa[0] = front_a(0)
        warmup(4)
        fa[1] = front_a(1)
        warmup(6)

        # ---- steady loop ----
        st_d_prev = None
        for ch in range(nch):
            st_b = front_b(ch, fa.pop(ch))
            if st_d_prev is not None:
                f_part(st_d_prev)
            st_v = v_part(st_b)
            if ch + 2 < nch:
                fa[ch + 2] = front_a(ch + 2)
            st_d_prev = dy_part(st_v)
        f_part(st_d_prev)

    nc.compile()
    return nc


def _pack_k(m: np.ndarray) -> np.ndarray:
    """(1024, F) contraction-major -> (128, 8*F); tile kt at [:, kt*F:(kt+1)*F]."""
    kdim, f = m.shape
    assert kdim == H
    return np.ascontiguousarray(
        m.reshape(JT, 128, f).transpose(1, 0, 2).reshape(128, JT * f))


def _pack_k_outer(m: np.ndarray) -> np.ndarray:
    """(1024, 1024) contraction-major -> (128, 8*1024) with the OUTPUT tile
    index outer: tile (kt, jt) at [:, jt*1024 + kt*128]."""
    t = m.reshape(JT, 128, JT, 128).transpose(1, 2, 0, 3)  # (kp, jt, kt, jc)
    return np.ascontiguousarray(t.reshape(128, JT * H))


def _f16(a):
    return np.ascontiguousarray(a).astype(np.float16)


def prep_weights(w1, w1_2, b1, w2, b2, w3, b3):
    f = np.float32
    wh1 = np.stack([w1[:, 0], w1[:, 1], w1_2[:, 0], w1_2[:, 1],
                    b1]).astype(f)                          # (5, 1024)
    wh2 = _pack_k_outer(np.ascontiguousarray(w2.T).astype(f))
    wv = _pack_k_outer(w2.astype(f))
    wy = np.zeros((128, JT), f)
    wy[:, :] = w3.reshape(JT, 128).T
    wd = _pack_k(np.ascontiguousarray(
        np.stack([w1[:, 0], w1_2[:, 0], w1[:, 1], w1_2[:, 1]], axis=1)).astype(f))
    wf = np.zeros((128, 3), f)
    wf[[0, 1, 32, 33], 0] = 1.0
    wf[[2, 3, 34, 35], 1] = -1.0
    wf[[64, 96], 2] = 1.0
    # h2 consumes e1 = z1sq - 1, so fold w2 @ 1 into the bias
    b2p = (b2.astype(f) + w2.astype(f).sum(axis=1))
    b2t = np.ascontiguousarray(b2p.reshape(JT, 128).T)
    b3vv = np.zeros((4, 1), f)
    # y consumes e2 = z2sq - 1, so fold w3 @ 1 into the bias
    b3vv[2, 0] = (np.asarray(b3, dtype=f).reshape(-1)[0]
                  + w3.astype(f).sum())
    w3s = np.ascontiguousarray((-4.0 * w3.reshape(H)).reshape(JT, 128).T.astype(f))
    return dict(wh1=_f16(wh1), wh2=_f16(wh2), wv=_f16(wv), wy=_f16(wy),
                wd=_f16(wd), wf=_f16(wf), b2t=b2t, b3v=b3vv, w3s=w3s)


_PROG_CACHE: dict = {}


def _install_trace_support():
    """The agent image lacks the ``antenv.axon_hooks`` shim that the axon
    NTFF-profiling path imports; recreate it and register the ctypes hook.
    Also neuter ``upload_artifacts`` (zero-egress container)."""
    import sys
    import types
    try:
        import antenv.axon_hooks  # noqa: F401
    except ImportError:
        import antenv
        mod = types.ModuleType("antenv.axon_hooks")
        holder = {}
        mod.set_axon_ntff_profile_hook = lambda h: holder.__setitem__("h", h)
        mod.get_axon_ntff_profile_hook = lambda: holder.get("h")
        sys.modules["antenv.axon_hooks"] = mod
        antenv.axon_hooks = mod
        from trn_agent_boot.trn_boot import _ntff_profile_via_ctypes
        hook = _ntff_profile_via_ctypes("/opt/axon/libaxon_pjrt.so")
        if hook is not None:
            mod.set_axon_ntff_profile_hook(hook)
    import concourse.bass_utils as bu
    bu.upload_artifacts = lambda tmpdir: tmpdir


def kernel(x, w1, w1_2, b1, w2, b2, w3, b3, trace=False, _chunk=512):
    x = np.asarray(x, dtype=np.float32)
    wdict = prep_weights(np.asarray(w1), np.asarray(w1_2), np.asarray(b1),
                         np.asarray(w2), np.asarray(b2), np.asarray(w3),
                         np.asarray(b3))

    key = (NXL, _chunk)
    if key not in _PROG_CACHE:
        _PROG_CACHE[key] = build_program(NXL, _chunk)
    nc = _PROG_CACHE[key]

    in_maps = []
    ones = np.ones((NXL,), dtype=np.float32)
    for c in range(N_CORES):
        xs = x[c * NXL:(c + 1) * NXL]                 # (NXL, 2)
        x0, x1 = xs[:, 0].copy(), xs[:, 1].copy()
        xrs = _f16(np.stack([x0 * x0, x1 * x1, x0, x1, ones]))   # (5, NXL)
        xqs = _f16(np.stack([x0, ones, x1, ones]))               # (4, NXL)
        in_maps.append({"xr": xrs, "xq": xqs, **wdict})

    if trace:
        _install_trace_support()
    res = run_bass_kernel_spmd(nc, in_maps, core_ids=list(range(N_CORES)),
                               trace=trace)

    y = np.concatenate([res.results[c]["outy"].reshape(NXL)
                        for c in range(N_CORES)]).reshape(NX, 1)
    d2 = np.concatenate([res.results[c]["outd2"].reshape(NXL)
                         for c in range(N_CORES)]).reshape(NX, 1)
    m1 = np.concatenate([res.results[c]["outm1"].reshape(NXL)
                         for c in range(N_CORES)]).reshape(NX, 1)
    out = (y.astype(np.float32), d2.astype(np.float32), m1.astype(np.float32))
    if trace:
        return out, res
    return out


# revision 12
# speedup vs baseline: 1.1297x; 1.1297x over previous
"""Trainium2 Bass kernel for nn_DerivNet2D_v2 (quadratic-feature MLP fwd + 2
directional derivatives).

Math (feature-major, per sample n):
  h1 = W5 @ [x0^2; x1^2; x0; x1; 1]             (1024, nx)   b1 folded in
  z1 = tanh(h1);  e1 = z1^2 - 1                 (= -sech^2(h1))
  h2 = w2 @ e1 + (b2 + rowsum(w2));  z2 = tanh(h2)
  e2 = z2^2 - 1;  y = w3 @ e2 + (b3 + sum(w3))  (bias-fold of the +1)
  gt = e2 * (z2 * w3s),  w3s = -4*w3            (= G = 4 w3 z2 (1-z2^2))
  v  = w2^T-contraction: v[i,n] = sum_j w2[j,i] gt[j,n]
  qt = e1 * z1  (= -z1(1-z1^2));  qv = qt * v
  d-reduce: psd[0:4] = sum_{it<4} wd_it^T qv_it; psd[32:36] = sum_{it>=4}
  y-reduce: psd[64] = sum_{jt<4} w3_jt^T e2_jt; psd[96] = rest
  tt = psd * xx  (xx rows: {0-3,32-35}=[x0,1,x1,1], {64,96}=1, else 0)
  f  = wf^T @ tt -> [ -dydx1, dydx2, y-b3' ];  fs = f + [0,0,b3'] -> outputs

Engine placement: big matmuls (h2, v) + all reductions on PE; tanh/
final-copy on ACT; sq/e1/e2/zw/qv/tt on DVE; qt/gt on GPSIMD; SWDGE out.
Small matmuls packed with tile_position: h1 row-tiled 2x (K=5), d/y reduces
col-tiled 4x (M<=4).  Software pipeline: front_a (h1 stage) runs two chunks
ahead so the PE never waits on the ACT tanh queue at chunk boundaries.

Sharding: pure data-parallel over 8 cores along batch; weights replicated.
"""

import numpy as np
from contextlib import ExitStack

import concourse.bass as bass
import concourse.tile as tile
from concourse import bacc, mybir
from concourse.bass_utils import run_bass_kernel_spmd

F32 = mybir.dt.float32
F16 = mybir.dt.float16
AF = mybir.ActivationFunctionType
ALU = mybir.AluOpType

NX = 32768
H = 1024
N_CORES = 8
NXL = NX // N_CORES  # 4096 per core
JT = H // 128        # 8 feature tiles of 128

# Fallback flags (set True only if HW misbehaves)
NO_ROW_TILE_H1 = False   # serialize h1 matmuls in 128x128 mode
NO_COL_TILE_DY = False   # serialize d/y reduce matmuls
QT_ON_DVE = False        # compute qt on DVE instead of GPSIMD
GT_ON_DVE = False        # compute gt on DVE instead of GPSIMD


def build_program(nxl: int, C: int):
    nch = nxl // C
    nc = bacc.Bacc("TRN2", target_bir_lowering=False, debug=False,
                   enable_asserts=False)

    # ---- DRAM I/O ----
    xr = nc.dram_tensor("xr", (5, nxl), F16, kind="ExternalInput").ap()
    xq = nc.dram_tensor("xq", (4, nxl), F16, kind="ExternalInput").ap()
    wh1 = nc.dram_tensor("wh1", (5, JT * 128), F16, kind="ExternalInput").ap()
    wh2 = nc.dram_tensor("wh2", (128, JT * H), F16, kind="ExternalInput").ap()
    wv = nc.dram_tensor("wv", (128, JT * H), F16, kind="ExternalInput").ap()
    wy = nc.dram_tensor("wy", (128, JT), F16, kind="ExternalInput").ap()
    wd = nc.dram_tensor("wd", (128, 4 * JT), F16, kind="ExternalInput").ap()
    wf = nc.dram_tensor("wf", (128, 3), F16, kind="ExternalInput").ap()
    b2t = nc.dram_tensor("b2t", (128, JT), F32, kind="ExternalInput").ap()
    b3v = nc.dram_tensor("b3v", (4, 1), F32, kind="ExternalInput").ap()

    outy = nc.dram_tensor("outy", (1, nxl), F32, kind="ExternalOutput").ap()
    outd2 = nc.dram_tensor("outd2", (1, nxl), F32, kind="ExternalOutput").ap()
    outm1 = nc.dram_tensor("outm1", (1, nxl), F32, kind="ExternalOutput").ap()

    with tile.TileContext(nc) as tc, ExitStack() as ctx:
        # ---- persistent tiles ----
        wpool = ctx.enter_context(tc.tile_pool(name="weights", bufs=1))
        s_wh2 = wpool.tile([128, JT * H], F16, tag="wh2")
        s_wv = wpool.tile([128, JT * H], F16, tag="wv")
        s_wh1 = wpool.tile([128, JT * 128], F16, tag="wh1")
        s_wy = wpool.tile([128, JT], F16, tag="wy")
        s_wd = wpool.tile([128, 4 * JT], F16, tag="wd")
        s_wf = wpool.tile([128, 3], F16, tag="wf")
        s_b2 = wpool.tile([128, JT], F32, tag="b2")
        s_b3 = wpool.tile([4, 1], F32, tag="b3")
        s_one = wpool.tile([128, 512], F16, tag="one")
        r4 = wpool.tile([128, nxl], F16, tag="r4")
        xx = wpool.tile([128, nxl], F16, tag="xx")

        # prewarm the ACT table (tanh) off the critical path; all memsets
        # before the DMAs so the warmup matmuls can start immediately
        warm = wpool.tile([128, 16], F32, tag="warm")
        nc.gpsimd.memset(warm[:], 0.0)
        nc.scalar.activation(warm[:], warm[:], AF.Tanh)
        nc.gpsimd.memset(s_one[:], 1.0)
        nc.gpsimd.memset(xx[:], 0.0)
        nc.gpsimd.memset(xx[64:65, :], 1.0)
        nc.gpsimd.memset(xx[96:97, :], 1.0)

        # ---- input DMAs (emission order approximates priority) ----
        nc.sync.dma_start(r4[0:5, :], xr[:])
        nc.sync.dma_start(s_wh1[0:5, :], wh1[:])
        nc.sync.dma_start(s_b2[:], b2t[:])
        nc.sync.dma_start(s_wy[:], wy[:])
        nc.sync.dma_start(s_wd[:], wd[:])
        nc.sync.dma_start(s_wf[:], wf[:])
        nc.sync.dma_start(s_b3[:], b3v[:])
        # replicate x rows for the second h1 row-tile position
        nc.sync.dma_start(r4[64:69, :], r4[0:5, :])
        nc.sync.dma_start(s_wh1[64:69, :], s_wh1[0:5, :])
        for jt in range(JT):
            nc.sync.dma_start(s_wh2[:, jt * H:(jt + 1) * H],
                              wh2[:, jt * H:(jt + 1) * H])
        nc.sync.dma_start(xx[0:4, :], xq[:])
        nc.sync.dma_start(xx[32:36, :], xx[0:4, :])
        for it in range(JT):
            nc.sync.dma_start(s_wv[:, it * H:(it + 1) * H],
                              wv[:, it * H:(it + 1) * H])

        # ---- pools ----
        p_z1sq = ctx.enter_context(tc.tile_pool(name="z1sq", bufs=3))
        p_qt = ctx.enter_context(tc.tile_pool(name="qt", bufs=3))
        p_z2sq = ctx.enter_context(tc.tile_pool(name="z2sq", bufs=2))
        p_gt = ctx.enter_context(tc.tile_pool(name="gt", bufs=2))
        p_qv = ctx.enter_context(tc.tile_pool(name="qv", bufs=2))
        p_z1 = ctx.enter_context(tc.tile_pool(name="z1", bufs=10))
        p_z2 = ctx.enter_context(tc.tile_pool(name="z2", bufs=6))
        p_zw = ctx.enter_context(tc.tile_pool(name="zw", bufs=4))
        p_tt = ctx.enter_context(tc.tile_pool(name="tt", bufs=2))
        p_fs = ctx.enter_context(tc.tile_pool(name="fs", bufs=2))
        p_big = ctx.enter_context(tc.tile_pool(name="bigps", bufs=4, space="PSUM"))
        p_h1 = ctx.enter_context(tc.tile_pool(name="h1ps", bufs=2, space="PSUM"))
        p_dy = ctx.enter_context(tc.tile_pool(name="dyps", bufs=1, space="PSUM"))
        p_f = ctx.enter_context(tc.tile_pool(name="fps", bufs=1, space="PSUM"))

        # d/y reduction accumulator bank: col-tiled matmuls write partition
        # groups {0-3, 32-35, 64, 96}; everything else must stay 0 (the f
        # matmul reads all 128 partitions).
        psdy = p_dy.tile([128, C], F32, tag="psdy")
        nc.vector.memset(psdy[:], 0.0)

        # PE clock prewarm (HAM gate holds PE at 1.2 GHz for ~3.4us)
        wtile = wpool.tile([128, C], F16, tag="warmw")
        nc.gpsimd.memset(wtile[:], 0.0)
        def warmup(n):
            psw = p_big.tile([128, C], F32, tag="big")
            for _ in range(n):
                nc.tensor.matmul(psw[:], wtile[:, 0:128], wtile[:],
                                 start=True, stop=True)

        def front_a(ch):
            """h1 matmuls (row-tiled 2x) -> tanh -> e1 (DVE), qt (GPSIMD)."""
            cs = slice(ch * C, (ch + 1) * C)
            z1s = []
            if NO_ROW_TILE_H1:
                for kt in range(JT):
                    ps = p_h1.tile([128, C], F32, tag="h1")
                    nc.tensor.matmul(ps[:], s_wh1[0:5, kt * 128:(kt + 1) * 128],
                                     r4[0:5, cs], start=True, stop=True)
                    z1 = p_z1.tile([128, C], F16, tag="z1")
                    nc.scalar.activation(z1[:], ps[:], AF.Tanh)
                    z1s.append(z1)
            else:
                for w in range(4):
                    kt0, kt1 = 2 * w, 2 * w + 1
                    psA = p_h1.tile([128, C], F32, tag="h1")
                    psB = p_h1.tile([128, C], F32, tag="h1")
                    nc.tensor.matmul(psA[:], s_wh1[0:5, kt0 * 128:(kt0 + 1) * 128],
                                     r4[0:5, cs], start=True, stop=True,
                                     tile_position=(0, 0))
                    nc.tensor.matmul(psB[:], s_wh1[64:69, kt1 * 128:(kt1 + 1) * 128],
                                     r4[64:69, cs], start=True, stop=True,
                                     tile_position=(64, 0))
                    zA = p_z1.tile([128, C], F16, tag="z1")
                    zB = p_z1.tile([128, C], F16, tag="z1")
                    nc.scalar.activation(zA[:], psA[:], AF.Tanh)
                    nc.scalar.activation(zB[:], psB[:], AF.Tanh)
                    z1s += [zA, zB]
            e1 = p_z1sq.tile([128, JT * C], F16, tag="e1")
            qt = p_qt.tile([128, JT * C], F16, tag="qt")
            qt_eng = nc.vector if QT_ON_DVE else nc.gpsimd
            for kt in range(JT):
                ks = slice(kt * C, (kt + 1) * C)
                sq = p_zw.tile([128, C], F16, tag="zw")
                nc.vector.tensor_mul(sq[:], z1s[kt][:], z1s[kt][:])
                nc.vector.tensor_sub(e1[:, ks], sq[:], s_one[:, 0:C])
                qt_eng.tensor_mul(qt[:, ks], e1[:, ks], z1s[kt][:])
            return cs, e1, qt

        def front_b(ch, st_a):
            """h2 -> z2 -> e2, gt."""
            cs, e1, qt = st_a
            e2 = p_z2sq.tile([128, JT * C], F16, tag="e2")
            gt = p_gt.tile([128, JT * C], F16, tag="gt")
            gt_eng = nc.vector
            for jt in range(JT):
                js = slice(jt * C, (jt + 1) * C)
                ps = p_big.tile([128, C], F32, tag="big")
                for kt in range(JT):
                    nc.tensor.matmul(
                        ps[:],
                        s_wh2[:, jt * H + kt * 128:jt * H + (kt + 1) * 128],
                        e1[:, kt * C:(kt + 1) * C],
                        start=(kt == 0), stop=(kt == JT - 1))
                z2 = p_z2.tile([128, C], F16, tag="z2")
                nc.scalar.activation(z2[:], ps[:], AF.Tanh,
                                     bias=s_b2[:, jt:jt + 1])
                sq = p_zw.tile([128, C], F16, tag="zw")
                nc.scalar.activation(sq[:], z2[:], AF.Square)
                nc.vector.tensor_sub(e2[:, js], sq[:], s_one[:, 0:C])
                gt_eng.tensor_mul(gt[:, js], e2[:, js], z2[:])
            return cs, qt, gt, e2

        def v_part(st_b):
            """v matmuls (pair-interleaved) and qv."""
            cs, qt, gt, e2 = st_b
            qv = p_qv.tile([128, JT * C], F16, tag="qv")
            for p in range(4):
                it0, it1 = 2 * p, 2 * p + 1
                ps0 = p_big.tile([128, C], F32, tag="big")
                ps1 = p_big.tile([128, C], F32, tag="big")
                for jt in range(JT):
                    nc.tensor.matmul(
                        ps0[:],
                        s_wv[:, it0 * H + jt * 128:it0 * H + (jt + 1) * 128],
                        gt[:, jt * C:(jt + 1) * C],
                        start=(jt == 0), stop=(jt == JT - 1))
                    nc.tensor.matmul(
                        ps1[:],
                        s_wv[:, it1 * H + jt * 128:it1 * H + (jt + 1) * 128],
                        gt[:, jt * C:(jt + 1) * C],
                        start=(jt == 0), stop=(jt == JT - 1))
                nc.vector.tensor_mul(qv[:, it0 * C:(it0 + 1) * C],
                                     qt[:, it0 * C:(it0 + 1) * C], ps0[:])
                nc.vector.tensor_mul(qv[:, it1 * C:(it1 + 1) * C],
                                     qt[:, it1 * C:(it1 + 1) * C], ps1[:])
            return cs, qv, e2

        def dy_part(st_v):
            """col-tiled d/y reduces -> tt."""
            cs, qv, e2 = st_v
            if NO_COL_TILE_DY:
                for w in range(JT):
                    nc.tensor.matmul(psdy[0:4, :], s_wd[:, 4 * w:4 * w + 4],
                                     qv[:, w * C:(w + 1) * C],
                                     start=(w == 0), stop=(w == JT - 1),
                                     skip_group_check=True)
                for w in range(JT):
                    nc.tensor.matmul(psdy[64:65, :], s_wy[:, w:w + 1],
                                     e2[:, w * C:(w + 1) * C],
                                     start=(w == 0), stop=(w == JT - 1),
                                     skip_group_check=True)
            else:
                for w in range(4):
                    nc.tensor.matmul(psdy[0:4, :], s_wd[:, 4 * w:4 * w + 4],
                                     qv[:, w * C:(w + 1) * C],
                                     start=(w == 0), stop=(w == 3),
                                     tile_position=(0, 0),
                                     skip_group_check=True)
                    nc.tensor.matmul(psdy[32:36, :],
                                     s_wd[:, 4 * (4 + w):4 * (4 + w) + 4],
                                     qv[:, (4 + w) * C:(5 + w) * C],
                                     start=(w == 0), stop=(w == 3),
                                     tile_position=(0, 32),
                                     skip_group_check=True)
                    nc.tensor.matmul(psdy[64:65, :], s_wy[:, w:w + 1],
                                     e2[:, w * C:(w + 1) * C],
                                     start=(w == 0), stop=(w == 3),
                                     tile_position=(0, 64),
                                     skip_group_check=True)
                    nc.tensor.matmul(psdy[96:97, :], s_wy[:, 4 + w:5 + w],
                                     e2[:, (4 + w) * C:(5 + w) * C],
                                     start=(w == 0), stop=(w == 3),
                                     tile_position=(0, 96),
                                     skip_group_check=True)
            tt = p_tt.tile([128, C], F16, tag="tt")
            nc.vector.tensor_mul(tt[:], psdy[:], xx[:, cs])
            return cs, tt

        def f_part(st_d):
            """final combine matmul -> fs -> output DMAs."""
            cs, tt = st_d
            psf = p_f.tile([4, C], F32, tag="f")
            nc.tensor.matmul(psf[0:3, :], s_wf[:], tt[:],
                             start=True, stop=True)
            fs = p_fs.tile([3, C], F32, tag="fs")
            nc.scalar.activation(fs[:], psf[0:3, :], AF.Identity,
                                 bias=s_b3[0:3, 0:1])
            nc.gpsimd.dma_start(outm1[0:1, cs], fs[0:1, :])
            nc.gpsimd.dma_start(outd2[0:1, cs], fs[1:2, :])
            nc.gpsimd.dma_start(outy[0:1, cs], fs[2:3, :])

        # ---- prologue: warmup + h1 for chunks 0,1 ----
        warmup(4)
        fa = {}
        fa[0] = front_a(0)
        warmup(4)
        fa[1] = front_a(1)
        warmup(6)

        # ---- steady loop ----
        st_d_prev = None
        for ch in range(nch):
            st_b = front_b(ch, fa.pop(ch))
            if st_d_prev is not None:
                f_part(st_d_prev)
            st_v = v_part(st_b)
            if ch + 2 < nch:
                fa[ch + 2] = front_a(ch + 2)
            st_d_prev = dy_part(st_v)
        f_part(st_d_prev)

    nc.compile()
    return nc


def _pack_k(m: np.ndarray) -> np.ndarray:
    """(1024, F) contraction-major -> (128, 8*F); tile kt at [:, kt*F:(kt+1)*F]."""
    kdim, f = m.shape
    assert kdim == H
    return np.ascontiguousarray(
        m.reshape(JT, 128, f).transpose(1, 0, 2).reshape(128, JT * f))


def _pack_k_outer(m: np.ndarray) -> np.ndarray:
    """(1024, 1024) contraction-major -> (128, 8*1024) with the OUTPUT tile
    index outer: tile (kt, jt) at [:, jt*1024 + kt*128]."""
    t = m.reshape(JT, 128, JT, 128).transpose(1, 2, 0, 3)  # (kp, jt, kt, jc)
    return np.ascontiguousarray(t.reshape(128, JT * H))


def _f16(a):
    return np.ascontiguousarray(a).astype(np.float16)


def prep_weights(w1, w1_2, b1, w2, b2, w3, b3):
    f = np.float32
    wh1 = np.stack([w1[:, 0], w1[:, 1], w1_2[:, 0], w1_2[:, 1],
                    b1]).astype(f)                          # (5, 1024)
    wh2 = _pack_k_outer(np.ascontiguousarray(w2.T).astype(f))
    # fold gt's per-j scale (-4*w3[j]) into the v-matmul weights
    wv = _pack_k_outer((w2.astype(f)
                        * (-4.0 * w3.reshape(H, 1).astype(f))))
    wy = np.zeros((128, JT), f)
    wy[:, :] = w3.reshape(JT, 128).T
    wd = _pack_k(np.ascontiguousarray(
        np.stack([w1[:, 0], w1_2[:, 0], w1[:, 1], w1_2[:, 1]], axis=1)).astype(f))
    wf = np.zeros((128, 3), f)
    wf[[0, 1, 32, 33], 0] = 1.0
    wf[[2, 3, 34, 35], 1] = -1.0
    wf[[64, 96], 2] = 1.0
    # h2 consumes e1 = z1sq - 1, so fold w2 @ 1 into the bias
    b2p = (b2.astype(f) + w2.astype(f).sum(axis=1))
    b2t = np.ascontiguousarray(b2p.reshape(JT, 128).T)
    b3vv = np.zeros((4, 1), f)
    # y consumes e2 = z2sq - 1, so fold w3 @ 1 into the bias
    b3vv[2, 0] = (np.asarray(b3, dtype=f).reshape(-1)[0]
                  + w3.astype(f).sum())
    return dict(wh1=_f16(wh1), wh2=_f16(wh2), wv=_f16(wv), wy=_f16(wy),
                wd=_f16(wd), wf=_f16(wf), b2t=b2t, b3v=b3vv)


_PROG_CACHE: dict = {}


def _install_trace_support():
    """The agent image lacks the ``antenv.axon_hooks`` shim that the axon
    NTFF-profiling path imports; recreate it and register the ctypes hook.
    Also neuter ``upload_artifacts`` (zero-egress container)."""
    import sys
    import types
    try:
        import antenv.axon_hooks  # noqa: F401
    except ImportError:
        import antenv
        mod = types.ModuleType("antenv.axon_hooks")
        holder = {}
        mod.set_axon_ntff_profile_hook = lambda h: holder.__setitem__("h", h)
        mod.get_axon_ntff_profile_hook = lambda: holder.get("h")
        sys.modules["antenv.axon_hooks"] = mod
        antenv.axon_hooks = mod
        from trn_agent_boot.trn_boot import _ntff_profile_via_ctypes
        hook = _ntff_profile_via_ctypes("/opt/axon/libaxon_pjrt.so")
        if hook is not None:
            mod.set_axon_ntff_profile_hook(hook)
    import concourse.bass_utils as bu
    bu.upload_artifacts = lambda tmpdir: tmpdir


def kernel(x, w1, w1_2, b1, w2, b2, w3, b3, trace=False, _chunk=512):
    x = np.asarray(x, dtype=np.float32)
    wdict = prep_weights(np.asarray(w1), np.asarray(w1_2), np.asarray(b1),
                         np.asarray(w2), np.asarray(b2), np.asarray(w3),
                         np.asarray(b3))

    key = (NXL, _chunk)
    if key not in _PROG_CACHE:
        _PROG_CACHE[key] = build_program(NXL, _chunk)
    nc = _PROG_CACHE[key]

    in_maps = []
    ones = np.ones((NXL,), dtype=np.float32)
    for c in range(N_CORES):
        xs = x[c * NXL:(c + 1) * NXL]                 # (NXL, 2)
        x0, x1 = xs[:, 0].copy(), xs[:, 1].copy()
        xrs = _f16(np.stack([x0 * x0, x1 * x1, x0, x1, ones]))   # (5, NXL)
        xqs = _f16(np.stack([x0, ones, x1, ones]))               # (4, NXL)
        in_maps.append({"xr": xrs, "xq": xqs, **wdict})

    if trace:
        _install_trace_support()
    res = run_bass_kernel_spmd(nc, in_maps, core_ids=list(range(N_CORES)),
                               trace=trace)

    y = np.concatenate([res.results[c]["outy"].reshape(NXL)
                        for c in range(N_CORES)]).reshape(NX, 1)
    d2 = np.concatenate([res.results[c]["outd2"].reshape(NXL)
                         for c in range(N_CORES)]).reshape(NX, 1)
    m1 = np.concatenate([res.results[c]["outm1"].reshape(NXL)
                         for c in range(N_CORES)]).reshape(NX, 1)
    out = (y.astype(np.float32), d2.astype(np.float32), m1.astype(np.float32))
    if trace:
        return out, res
    return out


# revision 13
# speedup vs baseline: 1.1613x; 1.0279x over previous
"""Trainium2 Bass kernel for nn_DerivNet2D_v2 (quadratic-feature MLP fwd + 2
directional derivatives).

Math (feature-major, per sample n):
  h1 = W5 @ [x0^2; x1^2; x0; x1; 1]             (1024, nx)   b1 folded in
  z1 = tanh(h1);  e1 = z1^2 - 1                 (= -sech^2(h1))
  h2 = w2 @ e1 + (b2 + rowsum(w2));  z2 = tanh(h2)
  e2 = z2^2 - 1;  y = w3 @ e2 + (b3 + sum(w3))  (bias-fold of the +1)
  gt = e2 * (z2 * w3s),  w3s = -4*w3            (= G = 4 w3 z2 (1-z2^2))
  v  = w2^T-contraction: v[i,n] = sum_j w2[j,i] gt[j,n]
  qt = e1 * z1  (= -z1(1-z1^2));  qv = qt * v
  d-reduce: psd[0:4] = sum_{it<4} wd_it^T qv_it; psd[32:36] = sum_{it>=4}
  y-reduce: psd[64] = sum_{jt<4} w3_jt^T e2_jt; psd[96] = rest
  tt = psd * xx  (xx rows: {0-3,32-35}=[x0,1,x1,1], {64,96}=1, else 0)
  f  = wf^T @ tt -> [ -dydx1, dydx2, y-b3' ];  fs = f + [0,0,b3'] -> outputs

Engine placement: big matmuls (h2, v) + all reductions on PE; tanh/
final-copy on ACT; sq/e1/e2/zw/qv/tt on DVE; qt/gt on GPSIMD; SWDGE out.
Small matmuls packed with tile_position: h1 row-tiled 2x (K=5), d/y reduces
col-tiled 4x (M<=4).  Software pipeline: front_a (h1 stage) runs two chunks
ahead so the PE never waits on the ACT tanh queue at chunk boundaries.

Sharding: pure data-parallel over 8 cores along batch; weights replicated.
"""

import numpy as np
from contextlib import ExitStack

import concourse.bass as bass
import concourse.tile as tile
from concourse import bacc, mybir
from concourse.bass_utils import run_bass_kernel_spmd

F32 = mybir.dt.float32
F16 = mybir.dt.float16
AF = mybir.ActivationFunctionType
ALU = mybir.AluOpType

NX = 32768
H = 1024
N_CORES = 8
NXL = NX // N_CORES  # 4096 per core
JT = H // 128        # 8 feature tiles of 128

# Fallback flags (set True only if HW misbehaves)
NO_ROW_TILE_H1 = False   # serialize h1 matmuls in 128x128 mode
NO_COL_TILE_DY = False   # serialize d/y reduce matmuls
QT_ON_DVE = False        # compute qt on DVE instead of GPSIMD
GT_ON_DVE = False        # compute gt on DVE instead of GPSIMD


def build_program(nxl: int, C: int):
    nch = nxl // C
    nc = bacc.Bacc("TRN2", target_bir_lowering=False, debug=False,
                   enable_asserts=False)

    # ---- DRAM I/O ----
    xr = nc.dram_tensor("xr", (5, nxl), F16, kind="ExternalInput").ap()
    xq = nc.dram_tensor("xq", (4, nxl), F16, kind="ExternalInput").ap()
    wh1 = nc.dram_tensor("wh1", (5, JT * 128), F16, kind="ExternalInput").ap()
    wh2 = nc.dram_tensor("wh2", (128, JT * H), F16, kind="ExternalInput").ap()
    wv = nc.dram_tensor("wv", (128, JT * H), F16, kind="ExternalInput").ap()
    wy = nc.dram_tensor("wy", (128, JT), F16, kind="ExternalInput").ap()
    wd = nc.dram_tensor("wd", (128, 4 * JT), F16, kind="ExternalInput").ap()
    wf = nc.dram_tensor("wf", (128, 3), F16, kind="ExternalInput").ap()
    b2t = nc.dram_tensor("b2t", (128, JT), F32, kind="ExternalInput").ap()
    b3v = nc.dram_tensor("b3v", (4, 1), F32, kind="ExternalInput").ap()

    outy = nc.dram_tensor("outy", (1, nxl), F32, kind="ExternalOutput").ap()
    outd2 = nc.dram_tensor("outd2", (1, nxl), F32, kind="ExternalOutput").ap()
    outm1 = nc.dram_tensor("outm1", (1, nxl), F32, kind="ExternalOutput").ap()

    with tile.TileContext(nc) as tc, ExitStack() as ctx:
        # ---- persistent tiles ----
        wpool = ctx.enter_context(tc.tile_pool(name="weights", bufs=1))
        s_wh2 = wpool.tile([128, JT * H], F16, tag="wh2")
        s_wv = wpool.tile([128, JT * H], F16, tag="wv")
        s_wh1 = wpool.tile([128, JT * 128], F16, tag="wh1")
        s_wy = wpool.tile([128, JT], F16, tag="wy")
        s_wd = wpool.tile([128, 4 * JT], F16, tag="wd")
        s_wf = wpool.tile([128, 3], F16, tag="wf")
        s_b2 = wpool.tile([128, JT], F32, tag="b2")
        s_b3 = wpool.tile([4, 1], F32, tag="b3")
        s_one = wpool.tile([128, 512], F16, tag="one")
        r4 = wpool.tile([128, nxl], F16, tag="r4")
        xx = wpool.tile([128, nxl], F16, tag="xx")

        # prewarm the ACT table (tanh) off the critical path; all memsets
        # before the DMAs so the warmup matmuls can start immediately
        warm = wpool.tile([128, 16], F32, tag="warm")
        # PE clock prewarm tile (HAM gate holds PE at 1.2 GHz for ~3.4us)
        wtile = wpool.tile([128, C], F16, tag="warmw")
        nc.gpsimd.memset(wtile[:], 0.0)
        nc.gpsimd.memset(warm[:], 0.0)
        nc.scalar.activation(warm[:], warm[:], AF.Tanh)
        nc.gpsimd.memset(s_one[:], 1.0)
        nc.vector.memset(xx[:], 0.0)
        nc.gpsimd.memset(xx[64:65, :], 1.0)
        nc.gpsimd.memset(xx[96:97, :], 1.0)

        # ---- input DMAs (emission order approximates priority) ----
        nc.sync.dma_start(r4[0:5, :], xr[:])
        nc.sync.dma_start(r4[64:69, :], r4[0:5, :])
        nc.sync.dma_start(s_wh1[0:5, :], wh1[:])
        nc.sync.dma_start(s_wh1[64:69, :], s_wh1[0:5, :])
        nc.sync.dma_start(s_b2[:], b2t[:])
        nc.sync.dma_start(s_wy[:], wy[:])
        nc.sync.dma_start(s_wd[:], wd[:])
        nc.sync.dma_start(s_wf[:], wf[:])
        nc.sync.dma_start(s_b3[:], b3v[:])
        for jt in range(JT):
            nc.sync.dma_start(s_wh2[:, jt * H:(jt + 1) * H],
                              wh2[:, jt * H:(jt + 1) * H])
        nc.sync.dma_start(xx[0:4, :], xq[:])
        nc.sync.dma_start(xx[32:36, :], xx[0:4, :])
        for it in range(JT):
            nc.sync.dma_start(s_wv[:, it * H:(it + 1) * H],
                              wv[:, it * H:(it + 1) * H])

        # ---- pools ----
        p_z1sq = ctx.enter_context(tc.tile_pool(name="z1sq", bufs=3))
        p_qt = ctx.enter_context(tc.tile_pool(name="qt", bufs=3))
        p_z2sq = ctx.enter_context(tc.tile_pool(name="z2sq", bufs=2))
        p_gt = ctx.enter_context(tc.tile_pool(name="gt", bufs=2))
        p_qv = ctx.enter_context(tc.tile_pool(name="qv", bufs=2))
        p_z1 = ctx.enter_context(tc.tile_pool(name="z1", bufs=10))
        p_z2 = ctx.enter_context(tc.tile_pool(name="z2", bufs=6))
        p_zw = ctx.enter_context(tc.tile_pool(name="zw", bufs=4))
        p_tt = ctx.enter_context(tc.tile_pool(name="tt", bufs=2))
        p_fs = ctx.enter_context(tc.tile_pool(name="fs", bufs=2))
        p_big = ctx.enter_context(tc.tile_pool(name="bigps", bufs=4, space="PSUM"))
        p_h1 = ctx.enter_context(tc.tile_pool(name="h1ps", bufs=2, space="PSUM"))
        p_dy = ctx.enter_context(tc.tile_pool(name="dyps", bufs=1, space="PSUM"))
        p_f = ctx.enter_context(tc.tile_pool(name="fps", bufs=1, space="PSUM"))

        # d/y reduction accumulator bank: col-tiled matmuls write partition
        # groups {0-3, 32-35, 64, 96}; everything else must stay 0 (the f
        # matmul reads all 128 partitions).
        psdy = p_dy.tile([128, C], F32, tag="psdy")
        nc.vector.memset(psdy[:], 0.0)

        def warmup(n):
            psw = p_big.tile([128, C], F32, tag="big")
            for _ in range(n):
                nc.tensor.matmul(psw[:], wtile[:, 0:128], wtile[:],
                                 start=True, stop=True)

        def front_a(ch):
            """h1 matmuls (row-tiled 2x) -> tanh -> e1 (DVE), qt (GPSIMD)."""
            cs = slice(ch * C, (ch + 1) * C)
            z1s = []
            if NO_ROW_TILE_H1:
                for kt in range(JT):
                    ps = p_h1.tile([128, C], F32, tag="h1")
                    nc.tensor.matmul(ps[:], s_wh1[0:5, kt * 128:(kt + 1) * 128],
                                     r4[0:5, cs], start=True, stop=True)
                    z1 = p_z1.tile([128, C], F16, tag="z1")
                    nc.scalar.activation(z1[:], ps[:], AF.Tanh)
                    z1s.append(z1)
            else:
                for w in range(4):
                    kt0, kt1 = 2 * w, 2 * w + 1
                    psA = p_h1.tile([128, C], F32, tag="h1")
                    psB = p_h1.tile([128, C], F32, tag="h1")
                    nc.tensor.matmul(psA[:], s_wh1[0:5, kt0 * 128:(kt0 + 1) * 128],
                                     r4[0:5, cs], start=True, stop=True,
                                     tile_position=(0, 0))
                    nc.tensor.matmul(psB[:], s_wh1[64:69, kt1 * 128:(kt1 + 1) * 128],
                                     r4[64:69, cs], start=True, stop=True,
                                     tile_position=(64, 0))
                    zA = p_z1.tile([128, C], F16, tag="z1")
                    zB = p_z1.tile([128, C], F16, tag="z1")
                    nc.scalar.activation(zA[:], psA[:], AF.Tanh)
                    nc.scalar.activation(zB[:], psB[:], AF.Tanh)
                    z1s += [zA, zB]
            e1 = p_z1sq.tile([128, JT * C], F16, tag="e1")
            qt = p_qt.tile([128, JT * C], F16, tag="qt")
            qt_eng = nc.vector if QT_ON_DVE else nc.gpsimd
            for kt in range(JT):
                ks = slice(kt * C, (kt + 1) * C)
                sq = p_zw.tile([128, C], F16, tag="zw")
                nc.vector.tensor_mul(sq[:], z1s[kt][:], z1s[kt][:])
                nc.vector.tensor_sub(e1[:, ks], sq[:], s_one[:, 0:C])
                qt_eng.tensor_mul(qt[:, ks], e1[:, ks], z1s[kt][:])
            return cs, e1, qt

        def front_b(ch, st_a):
            """h2 -> z2 -> e2, gt."""
            cs, e1, qt = st_a
            e2 = p_z2sq.tile([128, JT * C], F16, tag="e2")
            gt = p_gt.tile([128, JT * C], F16, tag="gt")
            gt_eng = nc.vector
            for jt in range(JT):
                js = slice(jt * C, (jt + 1) * C)
                ps = p_big.tile([128, C], F32, tag="big")
                for kt in range(JT):
                    nc.tensor.matmul(
                        ps[:],
                        s_wh2[:, jt * H + kt * 128:jt * H + (kt + 1) * 128],
                        e1[:, kt * C:(kt + 1) * C],
                        start=(kt == 0), stop=(kt == JT - 1))
                z2 = p_z2.tile([128, C], F16, tag="z2")
                nc.scalar.activation(z2[:], ps[:], AF.Tanh,
                                     bias=s_b2[:, jt:jt + 1])
                sq = p_zw.tile([128, C], F16, tag="zw")
                nc.scalar.activation(sq[:], z2[:], AF.Square)
                nc.vector.tensor_sub(e2[:, js], sq[:], s_one[:, 0:C])
                gt_eng.tensor_mul(gt[:, js], e2[:, js], z2[:])
            return cs, qt, gt, e2

        def v_part(st_b):
            """v matmuls (pair-interleaved) and qv."""
            cs, qt, gt, e2 = st_b
            qv = p_qv.tile([128, JT * C], F16, tag="qv")
            for p in range(4):
                it0, it1 = 2 * p, 2 * p + 1
                ps0 = p_big.tile([128, C], F32, tag="big")
                ps1 = p_big.tile([128, C], F32, tag="big")
                for jt in range(JT):
                    nc.tensor.matmul(
                        ps0[:],
                        s_wv[:, it0 * H + jt * 128:it0 * H + (jt + 1) * 128],
                        gt[:, jt * C:(jt + 1) * C],
                        start=(jt == 0), stop=(jt == JT - 1))
                    nc.tensor.matmul(
                        ps1[:],
                        s_wv[:, it1 * H + jt * 128:it1 * H + (jt + 1) * 128],
                        gt[:, jt * C:(jt + 1) * C],
                        start=(jt == 0), stop=(jt == JT - 1))
                nc.vector.tensor_mul(qv[:, it0 * C:(it0 + 1) * C],
                                     qt[:, it0 * C:(it0 + 1) * C], ps0[:])
                nc.vector.tensor_mul(qv[:, it1 * C:(it1 + 1) * C],
                                     qt[:, it1 * C:(it1 + 1) * C], ps1[:])
            return cs, qv, e2

        def dy_part(st_v):
            """col-tiled d/y reduces -> tt."""
            cs, qv, e2 = st_v
            if NO_COL_TILE_DY:
                for w in range(JT):
                    nc.tensor.matmul(psdy[0:4, :], s_wd[:, 4 * w:4 * w + 4],
                                     qv[:, w * C:(w + 1) * C],
                                     start=(w == 0), stop=(w == JT - 1),
                                     skip_group_check=True)
                for w in range(JT):
                    nc.tensor.matmul(psdy[64:65, :], s_wy[:, w:w + 1],
                                     e2[:, w * C:(w + 1) * C],
                                     start=(w == 0), stop=(w == JT - 1),
                                     skip_group_check=True)
            else:
                for w in range(4):
                    nc.tensor.matmul(psdy[0:4, :], s_wd[:, 4 * w:4 * w + 4],
                                     qv[:, w * C:(w + 1) * C],
                                     start=(w == 0), stop=(w == 3),
                                     tile_position=(0, 0),
                                     skip_group_check=True)
                    nc.tensor.matmul(psdy[32:36, :],
                                     s_wd[:, 4 * (4 + w):4 * (4 + w) + 4],
                                     qv[:, (4 + w) * C:(5 + w) * C],
                                     start=(w == 0), stop=(w == 3),
                                     tile_position=(0, 32),
                                     skip_group_check=True)
                    nc.tensor.matmul(psdy[64:65, :], s_wy[:, w:w + 1],
                                     e2[:, w * C:(w + 1) * C],
                                     start=(w == 0), stop=(w == 3),
                                     tile_position=(0, 64),
                                     skip_group_check=True)
                    nc.tensor.matmul(psdy[96:97, :], s_wy[:, 4 + w:5 + w],
                                     e2[:, (4 + w) * C:(5 + w) * C],
                                     start=(w == 0), stop=(w == 3),
                                     tile_position=(0, 96),
                                     skip_group_check=True)
            tt = p_tt.tile([128, C], F16, tag="tt")
            nc.vector.tensor_mul(tt[:], psdy[:], xx[:, cs])
            return cs, tt

        def f_part(st_d):
            """final combine matmul -> fs -> output DMAs."""
            cs, tt = st_d
            psf = p_f.tile([4, C], F32, tag="f")
            nc.tensor.matmul(psf[0:3, :], s_wf[:], tt[:],
                             start=True, stop=True)
            fs = p_fs.tile([3, C], F32, tag="fs")
            nc.scalar.activation(fs[:], psf[0:3, :], AF.Identity,
                                 bias=s_b3[0:3, 0:1])
            nc.gpsimd.dma_start(outm1[0:1, cs], fs[0:1, :])
            nc.gpsimd.dma_start(outd2[0:1, cs], fs[1:2, :])
            nc.gpsimd.dma_start(outy[0:1, cs], fs[2:3, :])

        # ---- prologue: warmup + h1 for chunks 0,1 ----
        warmup(4)
        fa = {}
        fa[0] = front_a(0)
        warmup(4)
        fa[1] = front_a(1)
        warmup(6)

        # ---- steady loop ----
        # Mode-contiguous PE blocks per iteration: [128x128: h2, f, v]
        # [col-tiled: dy] [row-tiled: h1(ch+2)] -- 3 mode switches/chunk.
        st_d_prev = None
        for ch in range(nch):
            st_b = front_b(ch, fa.pop(ch))
            if st_d_prev is not None:
                f_part(st_d_prev)
            st_v = v_part(st_b)
            st_d_prev = dy_part(st_v)
            if ch + 2 < nch:
                fa[ch + 2] = front_a(ch + 2)
        f_part(st_d_prev)

    nc.compile()
    return nc


def _pack_k(m: np.ndarray) -> np.ndarray:
    """(1024, F) contraction-major -> (128, 8*F); tile kt at [:, kt*F:(kt+1)*F]."""
    kdim, f = m.shape
    assert kdim == H
    return np.ascontiguousarray(
        m.reshape(JT, 128, f).transpose(1, 0, 2).reshape(128, JT * f))


def _pack_k_outer(m: np.ndarray) -> np.ndarray:
    """(1024, 1024) contraction-major -> (128, 8*1024) with the OUTPUT tile
    index outer: tile (kt, jt) at [:, jt*1024 + kt*128]."""
    t = m.reshape(JT, 128, JT, 128).transpose(1, 2, 0, 3)  # (kp, jt, kt, jc)
    return np.ascontiguousarray(t.reshape(128, JT * H))


def _f16(a):
    return np.ascontiguousarray(a).astype(np.float16)


def prep_weights(w1, w1_2, b1, w2, b2, w3, b3):
    f = np.float32
    wh1 = np.stack([w1[:, 0], w1[:, 1], w1_2[:, 0], w1_2[:, 1],
                    b1]).astype(f)                          # (5, 1024)
    wh2 = _pack_k_outer(np.ascontiguousarray(w2.T).astype(f))
    # fold gt's per-j scale (-4*w3[j]) into the v-matmul weights
    wv = _pack_k_outer((w2.astype(f)
                        * (-4.0 * w3.reshape(H, 1).astype(f))))
    wy = np.zeros((128, JT), f)
    wy[:, :] = w3.reshape(JT, 128).T
    wd = _pack_k(np.ascontiguousarray(
        np.stack([w1[:, 0], w1_2[:, 0], w1[:, 1], w1_2[:, 1]], axis=1)).astype(f))
    wf = np.zeros((128, 3), f)
    wf[[0, 1, 32, 33], 0] = 1.0
    wf[[2, 3, 34, 35], 1] = -1.0
    wf[[64, 96], 2] = 1.0
    # h2 consumes e1 = z1sq - 1, so fold w2 @ 1 into the bias
    b2p = (b2.astype(f) + w2.astype(f).sum(axis=1))
    b2t = np.ascontiguousarray(b2p.reshape(JT, 128).T)
    b3vv = np.zeros((4, 1), f)
    # y consumes e2 = z2sq - 1, so fold w3 @ 1 into the bias
    b3vv[2, 0] = (np.asarray(b3, dtype=f).reshape(-1)[0]
                  + w3.astype(f).sum())
    return dict(wh1=_f16(wh1), wh2=_f16(wh2), wv=_f16(wv), wy=_f16(wy),
                wd=_f16(wd), wf=_f16(wf), b2t=b2t, b3v=b3vv)


_PROG_CACHE: dict = {}


def _install_trace_support():
    """The agent image lacks the ``antenv.axon_hooks`` shim that the axon
    NTFF-profiling path imports; recreate it and register the ctypes hook.
    Also neuter ``upload_artifacts`` (zero-egress container)."""
    import sys
    import types
    try:
        import antenv.axon_hooks  # noqa: F401
    except ImportError:
        import antenv
        mod = types.ModuleType("antenv.axon_hooks")
        holder = {}
        mod.set_axon_ntff_profile_hook = lambda h: holder.__setitem__("h", h)
        mod.get_axon_ntff_profile_hook = lambda: holder.get("h")
        sys.modules["antenv.axon_hooks"] = mod
        antenv.axon_hooks = mod
        from trn_agent_boot.trn_boot import _ntff_profile_via_ctypes
        hook = _ntff_profile_via_ctypes("/opt/axon/libaxon_pjrt.so")
        if hook is not None:
            mod.set_axon_ntff_profile_hook(hook)
    import concourse.bass_utils as bu
    bu.upload_artifacts = lambda tmpdir: tmpdir


def kernel(x, w1, w1_2, b1, w2, b2, w3, b3, trace=False, _chunk=512):
    x = np.asarray(x, dtype=np.float32)
    wdict = prep_weights(np.asarray(w1), np.asarray(w1_2), np.asarray(b1),
                         np.asarray(w2), np.asarray(b2), np.asarray(w3),
                         np.asarray(b3))

    key = (NXL, _chunk)
    if key not in _PROG_CACHE:
        _PROG_CACHE[key] = build_program(NXL, _chunk)
    nc = _PROG_CACHE[key]

    in_maps = []
    ones = np.ones((NXL,), dtype=np.float32)
    for c in range(N_CORES):
        xs = x[c * NXL:(c + 1) * NXL]                 # (NXL, 2)
        x0, x1 = xs[:, 0].copy(), xs[:, 1].copy()
        xrs = _f16(np.stack([x0 * x0, x1 * x1, x0, x1, ones]))   # (5, NXL)
        xqs = _f16(np.stack([x0, ones, x1, ones]))               # (4, NXL)
        in_maps.append({"xr": xrs, "xq": xqs, **wdict})

    if trace:
        _install_trace_support()
    res = run_bass_kernel_spmd(nc, in_maps, core_ids=list(range(N_CORES)),
                               trace=trace)

    y = np.concatenate([res.results[c]["outy"].reshape(NXL)
                        for c in range(N_CORES)]).reshape(NX, 1)
    d2 = np.concatenate([res.results[c]["outd2"].reshape(NXL)
                         for c in range(N_CORES)]).reshape(NX, 1)
    m1 = np.concatenate([res.results[c]["outm1"].reshape(NXL)
                         for c in range(N_CORES)]).reshape(NX, 1)
    out = (y.astype(np.float32), d2.astype(np.float32), m1.astype(np.float32))
    if trace:
        return out, res
    return out


# revision 14
# speedup vs baseline: 1.1734x; 1.0104x over previous
"""Trainium2 Bass kernel for nn_DerivNet2D_v2 (quadratic-feature MLP fwd + 2
directional derivatives).

Math (feature-major, per sample n):
  h1 = W5 @ [x0^2; x1^2; x0; x1; 1]             (1024, nx)   b1 folded in
  z1 = tanh(h1);  e1 = z1^2 - 1                 (= -sech^2(h1))
  h2 = w2 @ e1 + (b2 + rowsum(w2));  z2 = tanh(h2)
  e2 = z2^2 - 1;  y = w3 @ e2 + (b3 + sum(w3))  (bias-fold of the +1)
  gt = e2 * (z2 * w3s),  w3s = -4*w3            (= G = 4 w3 z2 (1-z2^2))
  v  = w2^T-contraction: v[i,n] = sum_j w2[j,i] gt[j,n]
  qt = e1 * z1  (= -z1(1-z1^2));  qv = qt * v
  d-reduce: psd[0:4] = sum_{it<4} wd_it^T qv_it; psd[32:36] = sum_{it>=4}
  y-reduce: psd[64] = sum_{jt<4} w3_jt^T e2_jt; psd[96] = rest
  tt = psd * xx  (xx rows: {0-3,32-35}=[x0,1,x1,1], {64,96}=1, else 0)
  f  = wf^T @ tt -> [ -dydx1, dydx2, y-b3' ];  fs = f + [0,0,b3'] -> outputs

Engine placement: big matmuls (h2, v) + all reductions on PE; tanh/
final-copy on ACT; sq/e1/e2/zw/qv/tt on DVE; qt/gt on GPSIMD; SWDGE out.
Small matmuls packed with tile_position: h1 row-tiled 2x (K=5), d/y reduces
col-tiled 4x (M<=4).  Software pipeline: front_a (h1 stage) runs two chunks
ahead so the PE never waits on the ACT tanh queue at chunk boundaries.

Sharding: pure data-parallel over 8 cores along batch; weights replicated.
"""

import numpy as np
from contextlib import ExitStack

import concourse.bass as bass
import concourse.tile as tile
from concourse import bacc, mybir
from concourse.bass_utils import run_bass_kernel_spmd

F32 = mybir.dt.float32
F16 = mybir.dt.float16
AF = mybir.ActivationFunctionType
ALU = mybir.AluOpType

NX = 32768
H = 1024
N_CORES = 8
NXL = NX // N_CORES  # 4096 per core
JT = H // 128        # 8 feature tiles of 128

# Fallback flags (set True only if HW misbehaves)
NO_ROW_TILE_H1 = False   # serialize h1 matmuls in 128x128 mode
NO_COL_TILE_DY = False   # serialize d/y reduce matmuls
QT_ON_DVE = False        # compute qt on DVE instead of GPSIMD
GT_ON_DVE = False        # compute gt on DVE instead of GPSIMD


def build_program(nxl: int, C: int):
    nch = nxl // C
    nc = bacc.Bacc("TRN2", target_bir_lowering=False, debug=False,
                   enable_asserts=False)

    # ---- DRAM I/O ----
    xr = nc.dram_tensor("xr", (5, nxl), F16, kind="ExternalInput").ap()
    xq = nc.dram_tensor("xq", (4, nxl), F16, kind="ExternalInput").ap()
    wh1 = nc.dram_tensor("wh1", (5, JT * 128), F16, kind="ExternalInput").ap()
    wh2 = nc.dram_tensor("wh2", (128, JT * H), F16, kind="ExternalInput").ap()
    wv = nc.dram_tensor("wv", (128, JT * H), F16, kind="ExternalInput").ap()
    wy = nc.dram_tensor("wy", (128, JT), F16, kind="ExternalInput").ap()
    wd = nc.dram_tensor("wd", (128, 4 * JT), F16, kind="ExternalInput").ap()
    wf = nc.dram_tensor("wf", (128, 3), F16, kind="ExternalInput").ap()
    b2t = nc.dram_tensor("b2t", (128, JT), F32, kind="ExternalInput").ap()
    b3v = nc.dram_tensor("b3v", (4, 1), F32, kind="ExternalInput").ap()

    outy = nc.dram_tensor("outy", (1, nxl), F32, kind="ExternalOutput").ap()
    outd2 = nc.dram_tensor("outd2", (1, nxl), F32, kind="ExternalOutput").ap()
    outm1 = nc.dram_tensor("outm1", (1, nxl), F32, kind="ExternalOutput").ap()

    with tile.TileContext(nc) as tc, ExitStack() as ctx:
        # ---- persistent tiles ----
        wpool = ctx.enter_context(tc.tile_pool(name="weights", bufs=1))
        s_wh2 = wpool.tile([128, JT * H], F16, tag="wh2")
        s_wv = wpool.tile([128, JT * H], F16, tag="wv")
        s_wh1 = wpool.tile([128, JT * 128], F16, tag="wh1")
        s_wy = wpool.tile([128, JT], F16, tag="wy")
        s_wd = wpool.tile([128, 4 * JT], F16, tag="wd")
        s_wf = wpool.tile([128, 3], F16, tag="wf")
        s_b2 = wpool.tile([128, JT], F32, tag="b2")
        s_b3 = wpool.tile([4, 1], F32, tag="b3")
        s_one = wpool.tile([128, 512], F16, tag="one")
        r4 = wpool.tile([128, nxl], F16, tag="r4")
        xx = wpool.tile([128, nxl], F16, tag="xx")

        # prewarm the ACT table (tanh) off the critical path; all memsets
        # before the DMAs so the warmup matmuls can start immediately
        warm = wpool.tile([128, 16], F32, tag="warm")
        # PE clock prewarm tile (HAM gate holds PE at 1.2 GHz for ~3.4us)
        wtile = wpool.tile([128, C], F16, tag="warmw")
        nc.gpsimd.memset(wtile[:], 0.0)
        nc.gpsimd.memset(warm[:], 0.0)
        nc.scalar.activation(warm[:], warm[:], AF.Tanh)
        nc.gpsimd.memset(s_one[:], 1.0)
        nc.vector.memset(xx[:], 0.0)
        nc.gpsimd.memset(xx[64:65, :], 1.0)
        nc.gpsimd.memset(xx[96:97, :], 1.0)

        # ---- input DMAs (emission order approximates priority) ----
        nc.sync.dma_start(r4[0:5, :], xr[:])
        nc.sync.dma_start(r4[64:69, :], r4[0:5, :])
        nc.sync.dma_start(s_wh1[0:5, :], wh1[:])
        nc.sync.dma_start(s_wh1[64:69, :], s_wh1[0:5, :])
        nc.sync.dma_start(s_b2[:], b2t[:])
        nc.sync.dma_start(s_wy[:], wy[:])
        nc.sync.dma_start(s_wd[:], wd[:])
        nc.sync.dma_start(s_wf[:], wf[:])
        nc.sync.dma_start(s_b3[:], b3v[:])
        for jt in range(JT):
            nc.sync.dma_start(s_wh2[:, jt * H:(jt + 1) * H],
                              wh2[:, jt * H:(jt + 1) * H])
        nc.sync.dma_start(xx[0:4, :], xq[:])
        nc.sync.dma_start(xx[32:36, :], xx[0:4, :])
        for it in range(JT):
            nc.sync.dma_start(s_wv[:, it * H:(it + 1) * H],
                              wv[:, it * H:(it + 1) * H])

        # ---- pools ----
        p_z1sq = ctx.enter_context(tc.tile_pool(name="z1sq", bufs=3))
        p_qt = ctx.enter_context(tc.tile_pool(name="qt", bufs=3))
        p_z2sq = ctx.enter_context(tc.tile_pool(name="z2sq", bufs=2))
        p_gt = ctx.enter_context(tc.tile_pool(name="gt", bufs=2))
        p_qv = ctx.enter_context(tc.tile_pool(name="qv", bufs=2))
        p_z1 = ctx.enter_context(tc.tile_pool(name="z1", bufs=10))
        p_z2 = ctx.enter_context(tc.tile_pool(name="z2", bufs=6))
        p_zw = ctx.enter_context(tc.tile_pool(name="zw", bufs=4))
        p_tt = ctx.enter_context(tc.tile_pool(name="tt", bufs=2))
        p_fs = ctx.enter_context(tc.tile_pool(name="fs", bufs=2))
        # One shared 6-bank pool for h2/v/h1 matmul outputs: h1's slots are
        # then freed by long-past v-consumers instead of its own tanh chain,
        # decoupling the PE from the ACT queue position of tanh-z1.
        p_big = ctx.enter_context(tc.tile_pool(name="bigps", bufs=6, space="PSUM"))
        p_dy = ctx.enter_context(tc.tile_pool(name="dyps", bufs=1, space="PSUM"))
        p_f = ctx.enter_context(tc.tile_pool(name="fps", bufs=1, space="PSUM"))

        # d/y reduction accumulator bank: col-tiled matmuls write partition
        # groups {0-3, 32-35, 64, 96}; everything else must stay 0 (the f
        # matmul reads all 128 partitions).
        psdy = p_dy.tile([128, C], F32, tag="psdy")
        nc.vector.memset(psdy[:], 0.0)

        def warmup(n):
            psw = p_big.tile([128, C], F32, tag="big")
            for _ in range(n):
                nc.tensor.matmul(psw[:], wtile[:, 0:128], wtile[:],
                                 start=True, stop=True)

        def front_a(ch):
            """h1 matmuls (row-tiled 2x) -> tanh -> e1 (DVE), qt (GPSIMD)."""
            cs = slice(ch * C, (ch + 1) * C)
            z1s = []
            if NO_ROW_TILE_H1:
                for kt in range(JT):
                    ps = p_big.tile([128, C], F32, tag="big")
                    nc.tensor.matmul(ps[:], s_wh1[0:5, kt * 128:(kt + 1) * 128],
                                     r4[0:5, cs], start=True, stop=True)
                    z1 = p_z1.tile([128, C], F16, tag="z1")
                    nc.scalar.activation(z1[:], ps[:], AF.Tanh)
                    z1s.append(z1)
            else:
                for w in range(4):
                    kt0, kt1 = 2 * w, 2 * w + 1
                    psA = p_big.tile([128, C], F32, tag="big")
                    psB = p_big.tile([128, C], F32, tag="big")
                    nc.tensor.matmul(psA[:], s_wh1[0:5, kt0 * 128:(kt0 + 1) * 128],
                                     r4[0:5, cs], start=True, stop=True,
                                     tile_position=(0, 0))
                    nc.tensor.matmul(psB[:], s_wh1[64:69, kt1 * 128:(kt1 + 1) * 128],
                                     r4[64:69, cs], start=True, stop=True,
                                     tile_position=(64, 0))
                    zA = p_z1.tile([128, C], F16, tag="z1")
                    zB = p_z1.tile([128, C], F16, tag="z1")
                    nc.scalar.activation(zA[:], psA[:], AF.Tanh)
                    nc.scalar.activation(zB[:], psB[:], AF.Tanh)
                    z1s += [zA, zB]
            e1 = p_z1sq.tile([128, JT * C], F16, tag="e1")
            qt = p_qt.tile([128, JT * C], F16, tag="qt")
            qt_eng = nc.vector if QT_ON_DVE else nc.gpsimd
            for kt in range(JT):
                ks = slice(kt * C, (kt + 1) * C)
                sq = p_zw.tile([128, C], F16, tag="zw")
                nc.vector.tensor_mul(sq[:], z1s[kt][:], z1s[kt][:])
                nc.vector.tensor_sub(e1[:, ks], sq[:], s_one[:, 0:C])
                qt_eng.tensor_mul(qt[:, ks], e1[:, ks], z1s[kt][:])
            return cs, e1, qt

        def front_b(ch, st_a):
            """h2 -> z2 -> e2, gt."""
            cs, e1, qt = st_a
            e2 = p_z2sq.tile([128, JT * C], F16, tag="e2")
            gt = p_gt.tile([128, JT * C], F16, tag="gt")
            gt_eng = nc.vector
            for jt in range(JT):
                js = slice(jt * C, (jt + 1) * C)
                ps = p_big.tile([128, C], F32, tag="big")
                for kt in range(JT):
                    nc.tensor.matmul(
                        ps[:],
                        s_wh2[:, jt * H + kt * 128:jt * H + (kt + 1) * 128],
                        e1[:, kt * C:(kt + 1) * C],
                        start=(kt == 0), stop=(kt == JT - 1))
                z2 = p_z2.tile([128, C], F16, tag="z2")
                nc.scalar.activation(z2[:], ps[:], AF.Tanh,
                                     bias=s_b2[:, jt:jt + 1])
                sq = p_zw.tile([128, C], F16, tag="zw")
                nc.scalar.activation(sq[:], z2[:], AF.Square)
                nc.vector.tensor_sub(e2[:, js], sq[:], s_one[:, 0:C])
                gt_eng.tensor_mul(gt[:, js], e2[:, js], z2[:])
            return cs, qt, gt, e2

        def v_part(st_b):
            """v matmuls (pair-interleaved) and qv."""
            cs, qt, gt, e2 = st_b
            qv = p_qv.tile([128, JT * C], F16, tag="qv")
            for p in range(4):
                it0, it1 = 2 * p, 2 * p + 1
                ps0 = p_big.tile([128, C], F32, tag="big")
                ps1 = p_big.tile([128, C], F32, tag="big")
                for jt in range(JT):
                    nc.tensor.matmul(
                        ps0[:],
                        s_wv[:, it0 * H + jt * 128:it0 * H + (jt + 1) * 128],
                        gt[:, jt * C:(jt + 1) * C],
                        start=(jt == 0), stop=(jt == JT - 1))
                    nc.tensor.matmul(
                        ps1[:],
                        s_wv[:, it1 * H + jt * 128:it1 * H + (jt + 1) * 128],
                        gt[:, jt * C:(jt + 1) * C],
                        start=(jt == 0), stop=(jt == JT - 1))
                nc.vector.tensor_mul(qv[:, it0 * C:(it0 + 1) * C],
                                     qt[:, it0 * C:(it0 + 1) * C], ps0[:])
                nc.vector.tensor_mul(qv[:, it1 * C:(it1 + 1) * C],
                                     qt[:, it1 * C:(it1 + 1) * C], ps1[:])
            return cs, qv, e2

        def dy_part(st_v):
            """col-tiled d/y reduces -> tt."""
            cs, qv, e2 = st_v
            if NO_COL_TILE_DY:
                for w in range(JT):
                    nc.tensor.matmul(psdy[0:4, :], s_wd[:, 4 * w:4 * w + 4],
                                     qv[:, w * C:(w + 1) * C],
                                     start=(w == 0), stop=(w == JT - 1),
                                     skip_group_check=True)
                for w in range(JT):
                    nc.tensor.matmul(psdy[64:65, :], s_wy[:, w:w + 1],
                                     e2[:, w * C:(w + 1) * C],
                                     start=(w == 0), stop=(w == JT - 1),
                                     skip_group_check=True)
            else:
                for w in range(4):
                    nc.tensor.matmul(psdy[0:4, :], s_wd[:, 4 * w:4 * w + 4],
                                     qv[:, w * C:(w + 1) * C],
                                     start=(w == 0), stop=(w == 3),
                                     tile_position=(0, 0),
                                     skip_group_check=True)
                    nc.tensor.matmul(psdy[32:36, :],
                                     s_wd[:, 4 * (4 + w):4 * (4 + w) + 4],
                                     qv[:, (4 + w) * C:(5 + w) * C],
                                     start=(w == 0), stop=(w == 3),
                                     tile_position=(0, 32),
                                     skip_group_check=True)
                    nc.tensor.matmul(psdy[64:65, :], s_wy[:, w:w + 1],
                                     e2[:, w * C:(w + 1) * C],
                                     start=(w == 0), stop=(w == 3),
                                     tile_position=(0, 64),
                                     skip_group_check=True)
                    nc.tensor.matmul(psdy[96:97, :], s_wy[:, 4 + w:5 + w],
                                     e2[:, (4 + w) * C:(5 + w) * C],
                                     start=(w == 0), stop=(w == 3),
                                     tile_position=(0, 96),
                                     skip_group_check=True)
            tt = p_tt.tile([128, C], F16, tag="tt")
            nc.vector.tensor_mul(tt[:], psdy[:], xx[:, cs])
            return cs, tt

        def f_part(st_d):
            """final combine matmul -> fs -> output DMAs."""
            cs, tt = st_d
            psf = p_f.tile([4, C], F32, tag="f")
            nc.tensor.matmul(psf[0:3, :], s_wf[:], tt[:],
                             start=True, stop=True)
            fs = p_fs.tile([3, C], F32, tag="fs")
            nc.scalar.activation(fs[:], psf[0:3, :], AF.Identity,
                                 bias=s_b3[0:3, 0:1])
            nc.gpsimd.dma_start(outm1[0:1, cs], fs[0:1, :])
            nc.gpsimd.dma_start(outd2[0:1, cs], fs[1:2, :])
            nc.gpsimd.dma_start(outy[0:1, cs], fs[2:3, :])

        # ---- prologue: warmup + h1 for chunks 0,1 ----
        warmup(4)
        fa = {}
        fa[0] = front_a(0)
        warmup(4)
        fa[1] = front_a(1)
        warmup(6)

        # ---- steady loop ----
        # Mode-contiguous PE blocks per iteration: [128x128: h2, v]
        # [col-tiled: f, dy] [row-tiled: h1(ch+2)] -- 3 mode switches/chunk.
        st_d_prev = None
        for ch in range(nch):
            st_b = front_b(ch, fa.pop(ch))
            st_v = v_part(st_b)
            if st_d_prev is not None:
                f_part(st_d_prev)        # col-mode, rides inside the dy block
            st_d_prev = dy_part(st_v)
            if ch + 2 < nch:
                fa[ch + 2] = front_a(ch + 2)
        f_part(st_d_prev)

    nc.compile()
    return nc


def _pack_k(m: np.ndarray) -> np.ndarray:
    """(1024, F) contraction-major -> (128, 8*F); tile kt at [:, kt*F:(kt+1)*F]."""
    kdim, f = m.shape
    assert kdim == H
    return np.ascontiguousarray(
        m.reshape(JT, 128, f).transpose(1, 0, 2).reshape(128, JT * f))


def _pack_k_outer(m: np.ndarray) -> np.ndarray:
    """(1024, 1024) contraction-major -> (128, 8*1024) with the OUTPUT tile
    index outer: tile (kt, jt) at [:, jt*1024 + kt*128]."""
    t = m.reshape(JT, 128, JT, 128).transpose(1, 2, 0, 3)  # (kp, jt, kt, jc)
    return np.ascontiguousarray(t.reshape(128, JT * H))


def _f16(a):
    return np.ascontiguousarray(a).astype(np.float16)


def prep_weights(w1, w1_2, b1, w2, b2, w3, b3):
    f = np.float32
    wh1 = np.stack([w1[:, 0], w1[:, 1], w1_2[:, 0], w1_2[:, 1],
                    b1]).astype(f)                          # (5, 1024)
    wh2 = _pack_k_outer(np.ascontiguousarray(w2.T).astype(f))
    # fold gt's per-j scale (-4*w3[j]) into the v-matmul weights
    wv = _pack_k_outer((w2.astype(f)
                        * (-4.0 * w3.reshape(H, 1).astype(f))))
    wy = np.zeros((128, JT), f)
    wy[:, :] = w3.reshape(JT, 128).T
    wd = _pack_k(np.ascontiguousarray(
        np.stack([w1[:, 0], w1_2[:, 0], w1[:, 1], w1_2[:, 1]], axis=1)).astype(f))
    wf = np.zeros((128, 3), f)
    wf[[0, 1, 32, 33], 0] = 1.0
    wf[[2, 3, 34, 35], 1] = -1.0
    wf[[64, 96], 2] = 1.0
    # h2 consumes e1 = z1sq - 1, so fold w2 @ 1 into the bias
    b2p = (b2.astype(f) + w2.astype(f).sum(axis=1))
    b2t = np.ascontiguousarray(b2p.reshape(JT, 128).T)
    b3vv = np.zeros((4, 1), f)
    # y consumes e2 = z2sq - 1, so fold w3 @ 1 into the bias
    b3vv[2, 0] = (np.asarray(b3, dtype=f).reshape(-1)[0]
                  + w3.astype(f).sum())
    return dict(wh1=_f16(wh1), wh2=_f16(wh2), wv=_f16(wv), wy=_f16(wy),
                wd=_f16(wd), wf=_f16(wf), b2t=b2t, b3v=b3vv)


_PROG_CACHE: dict = {}


def _install_trace_support():
    """The agent image lacks the ``antenv.axon_hooks`` shim that the axon
    NTFF-profiling path imports; recreate it and register the ctypes hook.
    Also neuter ``upload_artifacts`` (zero-egress container)."""
    import sys
    import types
    try:
        import antenv.axon_hooks  # noqa: F401
    except ImportError:
        import antenv
        mod = types.ModuleType("antenv.axon_hooks")
        holder = {}
        mod.set_axon_ntff_profile_hook = lambda h: holder.__setitem__("h", h)
        mod.get_axon_ntff_profile_hook = lambda: holder.get("h")
        sys.modules["antenv.axon_hooks"] = mod
        antenv.axon_hooks = mod
        from trn_agent_boot.trn_boot import _ntff_profile_via_ctypes
        hook = _ntff_profile_via_ctypes("/opt/axon/libaxon_pjrt.so")
        if hook is not None:
            mod.set_axon_ntff_profile_hook(hook)
    import concourse.bass_utils as bu
    bu.upload_artifacts = lambda tmpdir: tmpdir


def kernel(x, w1, w1_2, b1, w2, b2, w3, b3, trace=False, _chunk=512):
    x = np.asarray(x, dtype=np.float32)
    wdict = prep_weights(np.asarray(w1), np.asarray(w1_2), np.asarray(b1),
                         np.asarray(w2), np.asarray(b2), np.asarray(w3),
                         np.asarray(b3))

    key = (NXL, _chunk)
    if key not in _PROG_CACHE:
        _PROG_CACHE[key] = build_program(NXL, _chunk)
    nc = _PROG_CACHE[key]

    in_maps = []
    ones = np.ones((NXL,), dtype=np.float32)
    for c in range(N_CORES):
        xs = x[c * NXL:(c + 1) * NXL]                 # (NXL, 2)
        x0, x1 = xs[:, 0].copy(), xs[:, 1].copy()
        xrs = _f16(np.stack([x0 * x0, x1 * x1, x0, x1, ones]))   # (5, NXL)
        xqs = _f16(np.stack([x0, ones, x1, ones]))               # (4, NXL)
        in_maps.append({"xr": xrs, "xq": xqs, **wdict})

    if trace:
        _install_trace_support()
    res = run_bass_kernel_spmd(nc, in_maps, core_ids=list(range(N_CORES)),
                               trace=trace)

    y = np.concatenate([res.results[c]["outy"].reshape(NXL)
                        for c in range(N_CORES)]).reshape(NX, 1)
    d2 = np.concatenate([res.results[c]["outd2"].reshape(NXL)
                         for c in range(N_CORES)]).reshape(NX, 1)
    m1 = np.concatenate([res.results[c]["outm1"].reshape(NXL)
                         for c in range(N_CORES)]).reshape(NX, 1)
    out = (y.astype(np.float32), d2.astype(np.float32), m1.astype(np.float32))
    if trace:
        return out, res
    return out
